# revision 1
# baseline (speedup 1.0000x reference)
"""Transformer decoder layer (causal self-attn + cross-attn + FFN, post-LN)
on 8 trn2 NeuronCores via Bass/Tile.

Sharding (core c = 4*b + j; b = batch, j = rank in the 4-core batch group):
  - self-attention: HEAD-sharded (4 heads/core, all 2048 tokens, causal) ->
    identical SPMD structure on every core.
  - wo after self-attn: computed for ALL tokens against this core's 256
    head-dims, then ReduceScatter(add) within the batch group -> each core
    receives its own 512-token slice of the full wo output.
  - everything else (LN, cross-attn queries/output, FFN): TOKEN-sharded.
  - cross-attn K/V: each core projects its 512-token slice of `encoding`;
    AllGather within the batch group.

Layouts: activations transposed [d on partitions, tokens on free]. Matmuls
bf16, fp32 PSUM. Softmax exp on ScalarE, fused 1/32 scale, no max-subtraction
(scores/32 ~ +-1). Denominator: ones column appended to V (M=65 AV matmul).
Causal masks: multiplicative bf16 masks on the 4 diagonal 128-key tiles of
each 512-query window, applied post-exp. LN stats via ones-matmuls; std via
exp(0.5*ln(var)); row broadcast across partitions via DRAM bounce.
"""
import os
import numpy as np
import ml_dtypes

import concourse.bass as bass
import concourse.mybir as mybir
import concourse.tile as tile
from concourse import bacc
from concourse.bass_utils import run_bass_kernel_spmd

F32 = mybir.dt.float32
BF16 = mybir.dt.bfloat16
AF = mybir.ActivationFunctionType
OP = mybir.AluOpType

B, S, D, DHID, H = 2, 2048, 1024, 4096, 16
NT = 512
HL = 4
EPS = 1e-6
SCALE = 1.0 / 32.0

_CACHE = {}
LAST_RESULT = None


def _bf(a):
    return np.ascontiguousarray(np.asarray(a).astype(ml_dtypes.bfloat16))


def _f32(a):
    return np.ascontiguousarray(np.asarray(a, dtype=np.float32))


def build_nc():
    nc = bacc.Bacc("TRN2", target_bir_lowering=False, debug=False, num_devices=8)

    def inp(name, shape, dt=BF16):
        return nc.dram_tensor(name, shape, dt, kind="ExternalInput").ap()

    xtf = inp("xtf", [D, S])
    xf32 = inp("xf32", [D, NT], F32)
    ekv = inp("ekv", [D, NT])
    wq_blk = inp("wq_blk", [D, HL * 64])
    wk_blk = inp("wk_blk", [D, HL * 64])
    wv_blk = inp("wv_blk", [D, HL * 64])
    wo_blk = inp("wo_blk", [HL * 64, D])
    wqTc = inp("wqTc", [D, D])
    wkTc = inp("wkTc", [D, D])
    wvTc = inp("wvTc", [D, D])
    woTc = inp("woTc", [D, D])
    w1T = inp("w1T", [D, DHID])
    w2T = inp("w2T", [DHID, D])
    b1v = inp("b1v", [DHID], F32)
    b2v = inp("b2v", [D], F32)
    lng = inp("lng", [3, D], F32)
    lnb = inp("lnb", [3, D], F32)
    masks = inp("masks", [4, 128, 512])
    out_d = nc.dram_tensor("out", [D, NT], F32, kind="ExternalOutput").ap()

    RG = [[0, 1, 2, 3], [4, 5, 6, 7]]

    with tile.TileContext(nc) as tc:
        with (
            tc.tile_pool(name="ps", bufs=1, space="PSUM") as ps,
            tc.tile_pool(name="ps3", bufs=3, space="PSUM") as ps3,
            tc.tile_pool(name="dram", bufs=1, space="DRAM") as dram,
            tc.tile_pool(name="drow", bufs=4, space="DRAM") as drow,
            tc.tile_pool(name="pers", bufs=1) as pers,
            tc.tile_pool(name="wts", bufs=2) as wts,
            tc.tile_pool(name="w1", bufs=1) as w1pool,
            tc.tile_pool(name="w2", bufs=2) as w2pool,
            tc.tile_pool(name="w3", bufs=2) as w3pool,
            tc.tile_pool(name="rows", bufs=1) as rows,
        ):
            # ---------- static small sbuf ----------
            ones1 = pers.tile([128, 1], BF16, tag="ones1")
            nc.vector.memset(ones1[:], 1.0)
            mask_sb = pers.tile([128, 4, 512], BF16, tag="mask")
            nc.sync.dma_start(mask_sb[:], masks.rearrange("r p q -> p r q"))
            g_sb = pers.tile([128, 3, 8], F32, tag="lng")
            nc.sync.dma_start(g_sb[:], lng.rearrange("l (dt p) -> p l dt", p=128))
            bta_sb = pers.tile([128, 3, 8], F32, tag="lnb")
            nc.sync.dma_start(bta_sb[:], lnb.rearrange("l (dt p) -> p l dt", p=128))
            b1_sb = pers.tile([128, 32], F32, tag="b1")
            nc.sync.dma_start(b1_sb[:], b1v.rearrange("(t p) -> p t", p=128))
            b2_sb = pers.tile([128, 8], F32, tag="b2")
            nc.sync.dma_start(b2_sb[:], b2v.rearrange("(t p) -> p t", p=128))

            h1f_dram = dram.tile([128, 8, 512], F32)
            h2f_dram = dram.tile([128, 8, 512], F32)

            # ---------- helpers ----------
            def mm_proj(out_sb, w_dram, rhs_sb, jt_count, wtag):
                ntw = out_sb.shape[-1]
                w_sb = wts.tile([128, 8, jt_count * 128], BF16, tag=wtag)
                nc.sync.dma_start(w_sb[:], w_dram.rearrange("(kt p) j -> p kt j", p=128))
                for jt in range(jt_count):
                    for tw in range(ntw // 512):
                        pt = ps3.tile([128, 512], F32, tag="sc")
                        for kt in range(8):
                            nc.tensor.matmul(
                                pt[:], w_sb[:, kt, 128 * jt:128 * jt + 128],
                                rhs_sb[:, kt, 512 * tw:512 * tw + 512],
                                start=(kt == 0), stop=(kt == 7))
                        nc.any.tensor_copy(
                            out_sb[:, jt, 512 * tw:512 * tw + 512], pt[:])

            def bcast_row(row_f32_ap, parts, tag="repA"):
                n = row_f32_ap.shape[-1]
                slot = drow.tile([1, n], F32, tag="rowb")
                nc.sync.dma_start(slot[:], row_f32_ap)
                rep = rows.tile([parts, n], F32, tag=tag)
                src = bass.AP(tensor=slot.tensor, offset=slot.offset,
                              ap=[[0, parts]] + [list(p) for p in slot.ap][1:])
                nc.sync.dma_start(rep[:], src)
                return rep

            def attention(n_heads, pair_data, attn_out, qcs, nkt_of, diag,
                          post_qc=None):
                """pair_data(p) -> (kt_pair [128, 2048], vhat_pair [128,16,2,65])"""
                pairs = {}
                for qc in range(qcs):
                    for p in range(n_heads // 2):
                        if p not in pairs:
                            pairs[p] = pair_data(p)
                        kt_pair, vhat_pair = pairs[p]
                        for m in range(2):
                            h = 2 * p + m
                            p0 = 64 * m
                            nkt = nkt_of(qc)
                            av = ps.tile([65, 512], F32, tag=f"av{m}")
                            for g in range(nkt // 2):
                                sc = ps3.tile([128, 2, 512], F32, tag="sc")
                                for i in range(2):
                                    kt = 2 * g + i
                                    nc.tensor.matmul(
                                        sc[:, i, :],
                                        kt_pair[p0:p0 + 64, 128 * kt:128 * kt + 128],
                                        attn_q[p0:p0 + 64, p, 512 * qc:512 * qc + 512],
                                        start=True, stop=True)
                                ex = w3pool.tile([128, 2, 512], BF16, tag=f"ex{m}")
                                nc.scalar.activation(ex[:], sc[:], AF.Exp, scale=SCALE)
                                if diag:
                                    for i in range(2):
                                        r = 2 * g + i - (nkt - 4)
                                        if 0 <= r < 4:
                                            nc.vector.tensor_tensor(
                                                ex[:, i, :], ex[:, i, :],
                                                mask_sb[:, r, :], OP.mult)
                                for i in range(2):
                                    kt = 2 * g + i
                                    nc.tensor.matmul(
                                        av[:], vhat_pair[:, kt, m, :], ex[:, i, :],
                                        start=(kt == 0), stop=(kt == nkt - 1))
                            row = rows.tile([1, 512], F32, tag="ra")
                            nc.vector.tensor_copy(row[:], av[64:65, :])
                            rec = rows.tile([1, 512], F32, tag="rb")
                            nc.vector.reciprocal_approx_fast(rec[:], row[:])
                            recR = bcast_row(rec[:], 64)
                            nc.vector.tensor_tensor(
                                attn_out[p0:p0 + 64, p, 512 * qc:512 * qc + 512],
                                av[0:64, :], recR[:], OP.mult)
                    if post_qc is not None:
                        post_qc(qc)

            def layernorm(resid_sb, ln_idx, h_bf, hf_dram):
                """in-place: resid_sb <- LN(resid_sb); also h_bf (bf16) and
                optional f32 copy to hf_dram."""
                rb = w1pool.tile([128, 8, 512], BF16, tag="w8a")
                nc.vector.tensor_copy(rb[:], resid_sb[:])
                sq = w1pool.tile([128, 8, 512], BF16, tag="w8b")
                nc.vector.tensor_tensor(sq[:], rb[:], rb[:], OP.mult)
                psum = ps.tile([1, 512], F32, tag="av0")
                psq = ps.tile([1, 512], F32, tag="av1")
                for kt in range(8):
                    nc.tensor.matmul(psum[:], ones1[:], rb[:, kt, :],
                                     start=(kt == 0), stop=(kt == 7))
                for kt in range(8):
                    nc.tensor.matmul(psq[:], ones1[:], sq[:, kt, :],
                                     start=(kt == 0), stop=(kt == 7))
                mean = rows.tile([1, 512], F32, tag="ra")
                nc.vector.tensor_scalar(mean[:], psum[:], 1.0 / D, None, OP.mult)
                var = rows.tile([1, 512], F32, tag="rb")
                nc.vector.tensor_tensor(var[:], psum[:], mean[:], OP.mult)
                nc.vector.tensor_tensor(var[:], psq[:], var[:], OP.subtract)
                nc.vector.tensor_scalar(var[:], var[:], 1.0 / (D - 1), None, OP.mult)
                lnv = rows.tile([1, 512], F32, tag="rc")
                nc.scalar.activation(lnv[:], var[:], AF.Ln)
                std = rows.tile([1, 512], F32, tag="rd")
                nc.scalar.activation(std[:], lnv[:], AF.Exp, scale=0.5)
                nc.vector.tensor_scalar(std[:], std[:], EPS, None, OP.add)
                r = rows.tile([1, 512], F32, tag="rc")
                nc.vector.reciprocal_approx_fast(r[:], std[:])
                mr = rows.tile([1, 512], F32, tag="rd")
                nc.vector.tensor_tensor(mr[:], mean[:], r[:], OP.mult)
                rR = bcast_row(r[:], 128, "repA")
                mrR = bcast_row(mr[:], 128, "repB")
                nc.vector.tensor_tensor(
                    resid_sb[:], resid_sb[:],
                    rR[:, None, :].to_broadcast((128, 8, 512)), OP.mult)
                nc.vector.tensor_tensor(
                    resid_sb[:], resid_sb[:],
                    mrR[:, None, :].to_broadcast((128, 8, 512)), OP.subtract)
                for dt in range(8):
                    nc.vector.tensor_scalar(
                        resid_sb[:, dt, :], resid_sb[:, dt, :],
                        g_sb[:, ln_idx, dt:dt + 1], bta_sb[:, ln_idx, dt:dt + 1],
                        OP.mult, OP.add)
                if h_bf is not None:
                    nc.vector.tensor_copy(h_bf[:], resid_sb[:])
                if hf_dram is not None:
                    nc.sync.dma_start(hf_dram[:], resid_sb[:])

            # ========== phase A: cross K/V shard -> AllGather ==========
            ekv_sb = pers.tile([128, 8, 512], BF16, tag="at8")
            nc.sync.dma_start(ekv_sb[:], ekv.rearrange("(kt p) t -> p kt t", p=128))
            ag2_in = dram.tile([2, 1024, 512], BF16)

            ktc_sh = w1pool.tile([128, 8, 512], BF16, tag="w8a")
            mm_proj(ktc_sh, wkTc, ekv_sb, 8, "w16")
            nc.sync.dma_start(ag2_in[0].rearrange("(kt p) t -> p kt t", p=128), ktc_sh[:])

            wvc_sb = wts.tile([128, 8, 1024], BF16, tag="w16")
            nc.sync.dma_start(wvc_sb[:], wvTc.rearrange("(kt p) j -> p kt j", p=128))
            vc_sh = w1pool.tile([128, 4, 1024], BF16, tag="w8b")
            for tt in range(4):
                for s in range(2):
                    pt = ps3.tile([128, 512], F32, tag="sc")
                    for kt in range(8):
                        nc.tensor.matmul(
                            pt[:], ekv_sb[:, kt, 128 * tt:128 * tt + 128],
                            wvc_sb[:, kt, 512 * s:512 * s + 512],
                            start=(kt == 0), stop=(kt == 7))
                    nc.any.tensor_copy(vc_sh[:, tt, 512 * s:512 * s + 512], pt[:])
            # region 1 flat == V natural [512 tok, 1024 d] row-major
            nc.sync.dma_start(
                ag2_in[1].rearrange("a t -> (a t)").rearrange(
                    "(tt p j) -> p tt j", p=128, j=1024),
                vc_sh.rearrange("p tt j -> p tt j"))
            ag2_out = dram.tile([4, 2, 1024, 512], BF16)
            nc.gpsimd.collective_compute(
                "AllGather", OP.bypass, replica_groups=RG,
                ins=[ag2_in[:].opt()], outs=[ag2_out[:].opt()])

            # ========== phase B: self QKV (head-block) ==========
            xtf_sb = pers.tile([128, 8, 2048], BF16, tag="big32")
            nc.sync.dma_start(xtf_sb[:], xtf.rearrange("(kt p) t -> p kt t", p=128))

            qt_s = pers.tile([128, 2, 2048], BF16, tag="qt8")
            mm_proj(qt_s, wq_blk, xtf_sb, 2, "w4s")
            kt_s = pers.tile([128, 2, 2048], BF16, tag="kb8")
            mm_proj(kt_s, wk_blk, xtf_sb, 2, "w4s")

            wvb_sb = wts.tile([128, 8, 256], BF16, tag="w4s")
            nc.sync.dma_start(wvb_sb[:], wv_blk.rearrange("(kt p) j -> p kt j", p=128))
            vhat_s = pers.tile([128, 16, HL, 65], BF16, tag="vh8")
            nc.vector.memset(vhat_s[:, :, :, 64:65], 1.0)
            for tt in range(16):
                pt = ps3.tile([128, 512], F32, tag="sc")
                for kt in range(8):
                    nc.tensor.matmul(
                        pt[:, 0:256], xtf_sb[:, kt, 128 * tt:128 * tt + 128],
                        wvb_sb[:, kt, :], start=(kt == 0), stop=(kt == 7))
                nc.any.tensor_copy(
                    vhat_s[:, tt, :, 0:64],
                    pt[:, 0:256].rearrange("p (h d) -> p h d", h=HL))

            # ========== phase C: self-attention ==========
            attnT = pers.tile([128, 2, 2048], BF16, tag="at8")
            attn_q = qt_s

            def self_pair(p):
                return kt_s[:, p, :], vhat_s[:, :, 2 * p:2 * p + 2, :]

            wob_sb = wts.tile([128, 2, 1024], BF16, tag="w4s")
            nc.sync.dma_start(wob_sb[:], wo_blk.rearrange("(kt p) j -> p kt j", p=128))
            rs_in = dram.tile([4, 1024, 512], BF16)

            def wo_partial(tc):
                for jt in range(8):
                    pt = ps3.tile([128, 512], F32, tag="sc")
                    for kt in range(2):
                        nc.tensor.matmul(
                            pt[:], wob_sb[:, kt, 128 * jt:128 * jt + 128],
                            attnT[:, kt, 512 * tc:512 * tc + 512],
                            start=(kt == 0), stop=(kt == 1))
                    ws = w3pool.tile([128, 512], BF16, tag=f"wo_cp{jt % 2}")
                    nc.any.tensor_copy(ws[:], pt[:])
                    nc.sync.dma_start(rs_in[tc, 128 * jt:128 * jt + 128, :], ws[:])

            attention(HL, self_pair, attnT, qcs=4,
                      nkt_of=lambda qc: 4 * (qc + 1), diag=True,
                      post_qc=wo_partial)
            rs_out = dram.tile([1024, 512], BF16)
            nc.gpsimd.collective_compute(
                "ReduceScatter", OP.add, replica_groups=RG,
                ins=[rs_in[:].opt()], outs=[rs_out[:].opt()])

            # cross_pair defined here so phase E can prefetch pair 0
            cross_pair_fn = []

            def cross_pair(p):
                ktp = w2pool.tile([128, 2048], BF16, tag="ktp")
                for r in range(4):
                    nc.sync.dma_start(
                        ktp[:, 512 * r:512 * r + 512],
                        ag2_out[r, 0, 128 * p:128 * p + 128, :])
                vhp = w2pool.tile([128, 16, 2, 65], BF16, tag="vhp")
                nc.vector.memset(vhp[:, :, :, 64:65], 1.0)
                for r in range(4):
                    src = ag2_out[r, 1].rearrange("a t -> (a t)").rearrange(
                        "(tt p hh dd) -> p tt hh dd", p=128, hh=H, dd=64)
                    for hh in range(2):
                        nc.sync.dma_start(
                            vhp[:, 4 * r:4 * r + 4, hh, 0:64],
                            src[:, :, 2 * p + hh, :])
                return ktp, vhp

            cross_pair_fn.append(cross_pair)

            # ========== phase E: resid1 + LN1 ==========
            sa_tok = w1pool.tile([128, 8, 512], BF16, tag="w8a")
            nc.sync.dma_start(sa_tok[:], rs_out.rearrange("(dt p) t -> p dt t", p=128))
            resid1 = w1pool.tile([128, 8, 512], F32, tag="residf")
            for dt in range(8):
                xj = w2pool.tile([128, 512], F32, tag="xj")
                nc.sync.dma_start(xj[:], xf32[128 * dt:128 * dt + 128, :])
                nc.vector.tensor_tensor(resid1[:, dt, :], xj[:], sa_tok[:, dt, :],
                                        OP.add)
            cross_prefetch = {0: cross_pair_fn[0](0)}
            h1b = pers.tile([128, 8, 512], BF16, tag="kb8")
            layernorm(resid1, 0, h1b, h1f_dram)

            # ========== phase F: cross Q ==========
            qt_c = pers.tile([128, 8, 512], BF16, tag="qt8")
            mm_proj(qt_c, wqTc, h1b, 8, "w16")
            attn_q = qt_c

            # ========== phase G: cross-attention ==========
            attnT2 = pers.tile([128, 8, 512], BF16, tag="at8")

            def cross_pair_cached(p):
                if cross_prefetch and p in cross_prefetch:
                    return cross_prefetch.pop(p)
                return cross_pair(p)

            attention(H, cross_pair_cached, attnT2, qcs=1,
                      nkt_of=lambda qc: 16, diag=False)

            # ========== phase H: cross wo + resid2 + LN2 ==========
            woc_sb = wts.tile([128, 8, 1024], BF16, tag="w16")
            nc.sync.dma_start(woc_sb[:], woTc.rearrange("(kt p) j -> p kt j", p=128))
            resid2 = w1pool.tile([128, 8, 512], F32, tag="residf")
            for jt in range(8):
                pt = ps3.tile([128, 512], F32, tag="sc")
                for kt in range(8):
                    nc.tensor.matmul(
                        pt[:], woc_sb[:, kt, 128 * jt:128 * jt + 128],
                        attnT2[:, kt, :], start=(kt == 0), stop=(kt == 7))
                hj = w2pool.tile([128, 512], F32, tag="xj")
                nc.sync.dma_start(hj[:], h1f_dram[:, jt, :])
                nc.vector.tensor_tensor(resid2[:, jt, :], pt[:], hj[:], OP.add)
            h2b = pers.tile([128, 8, 512], BF16, tag="vh8")
            layernorm(resid2, 1, h2b, h2f_dram)

            # ========== phase I: FFN + resid3 + LN3 -> out ==========
            zrelu = pers.tile([128, 32, 512], BF16, tag="big32")
            for hg in range(8):
                w1_sb = wts.tile([128, 8, 512], BF16, tag="w16")
                nc.sync.dma_start(
                    w1_sb[:],
                    w1T[:, 512 * hg:512 * hg + 512].rearrange(
                        "(kt p) j -> p kt j", p=128))
                for hh in range(4):
                    ht = 4 * hg + hh
                    pt = ps3.tile([128, 512], F32, tag="sc")
                    for kt in range(8):
                        nc.tensor.matmul(
                            pt[:], w1_sb[:, kt, 128 * hh:128 * hh + 128],
                            h2b[:, kt, :], start=(kt == 0), stop=(kt == 7))
                    nc.vector.tensor_scalar(
                        zrelu[:, ht, :], pt[:],
                        b1_sb[:, ht:ht + 1], 0.0, OP.add, OP.max)

            resid3 = w1pool.tile([128, 8, 512], F32, tag="residf")
            for jt in range(8):
                w2_sb = w2pool.tile([128, 32, 128], BF16, tag="w2s")
                nc.sync.dma_start(
                    w2_sb[:],
                    w2T[:, 128 * jt:128 * jt + 128].rearrange(
                        "(kt p) j -> p kt j", p=128))
                pt = ps3.tile([128, 512], F32, tag="sc")
                for kt in range(32):
                    nc.tensor.matmul(
                        pt[:], w2_sb[:, kt, :], zrelu[:, kt, :],
                        start=(kt == 0), stop=(kt == 31))
                hj = w2pool.tile([128, 512], F32, tag="xj")
                nc.sync.dma_start(hj[:], h2f_dram[:, jt, :])
                nc.vector.tensor_tensor(resid3[:, jt, :], pt[:], hj[:], OP.add)
                nc.vector.tensor_scalar(
                    resid3[:, jt, :], resid3[:, jt, :],
                    b2_sb[:, jt:jt + 1], None, OP.add)
            layernorm(resid3, 2, None, None)
            nc.sync.dma_start(out_d.rearrange("(dt p) t -> p dt t", p=128), resid3[:])

    nc.compile()
    return nc


def _host_prep(inputs):
    x = _f32(inputs["x"])
    enc = _f32(inputs["encoding"])
    wT = {k: _bf(np.asarray(inputs[k]).T) for k in
          ("sa_wq", "sa_wk", "sa_wv", "sa_wo", "ca_wq", "ca_wk", "ca_wv",
           "ca_wo", "ff_w1", "ff_w2")}
    lng = np.stack([_f32(inputs["ln1_g"]), _f32(inputs["ln2_g"]),
                    _f32(inputs["ln3_g"])])
    lnb = np.stack([_f32(inputs["ln1_b"]), _f32(inputs["ln2_b"]),
                    _f32(inputs["ln3_b"])])
    masks = np.zeros((4, 128, 512), np.float32)
    i = np.arange(128)[:, None]
    q = np.arange(512)[None, :]
    for r in range(4):
        masks[r] = (128 * r + i <= q).astype(np.float32)
    masks = _bf(masks)

    in_maps = []
    for c in range(8):
        b, j = c // 4, c % 4
        xT = np.ascontiguousarray(x[b].T)
        encT = np.ascontiguousarray(enc[b].T)
        sl = slice(NT * j, NT * (j + 1))
        hb = slice(256 * j, 256 * (j + 1))
        in_maps.append({
            "xtf": _bf(xT),
            "xf32": _f32(xT[:, sl]),
            "ekv": _bf(encT[:, sl]),
            "wq_blk": np.ascontiguousarray(wT["sa_wq"][:, hb]),
            "wk_blk": np.ascontiguousarray(wT["sa_wk"][:, hb]),
            "wv_blk": np.ascontiguousarray(wT["sa_wv"][:, hb]),
            "wo_blk": np.ascontiguousarray(wT["sa_wo"][hb, :]),
            "wqTc": wT["ca_wq"], "wkTc": wT["ca_wk"],
            "wvTc": wT["ca_wv"], "woTc": wT["ca_wo"],
            "w1T": wT["ff_w1"], "w2T": wT["ff_w2"],
            "b1v": _f32(inputs["ff_b1"]), "b2v": _f32(inputs["ff_b2"]),
            "lng": lng, "lnb": lnb, "masks": masks,
        })
    return in_maps


def kernel(**inputs):
    global LAST_RESULT
    if "nc" not in _CACHE:
        _CACHE["nc"] = build_nc()
    nc = _CACHE["nc"]
    in_maps = _host_prep(inputs)
    res = run_bass_kernel_spmd(nc, in_maps, list(range(8)),
                               trace=bool(os.environ.get("BASS_TRACE")))
    LAST_RESULT = res
    out = np.zeros((B, S, D), np.float32)
    for c in range(8):
        b, j = c // 4, c % 4
        out[b, NT * j:NT * (j + 1), :] = res.results[c]["out"].T
    return out

